# revision 1
# baseline (speedup 1.0000x reference)
"""Akima spline evaluation (nn_Akima_66623532696299) on 8 Trainium2 cores.

Strategy: data-parallel over the batch axis (8 batches per core). Per element
the spline y(x) is evaluated without any gather via the telescoped identity

    y(x) = v0 + sum_k g_k(d_k),   d_k = clamp(63*x - k, 0, 1)
    g_k(d) = d*(A_k + d*(B_k + d*C_k))

where g_k is segment k's cubic expressed in the normalized local coordinate
(g_k(1) = v_{k+1} - v_k exactly, so partial sums stay O(|v|) and fp32
accumulation is stable). Segment coefficients are derived from `value` on the
host in float64 and baked into the instruction stream as immediates.

Engines: DVE (vector) computes most segments' cubics with fused
tensor_scalar / scalar_tensor_tensor ops; GPSIMD computes a subset of
segments and owns the serial accumulation chain; ACT handles the 63*x
prescale; HWDGE (sync) does the DMA. The work split is chosen so DVE and
GPSIMD finish together.
"""

import numpy as np

N_CORES = 8
P = 128
B, CH, H, W = 64, 3, 512, 512
PER_CORE = (B // N_CORES) * CH * H * W        # 6291456
FTOT = PER_CORE // P                          # 49152
TF = 2048                                     # tile free size
NT = FTOT // TF                               # 24 tiles
NSEG = 63
GP_SEGS = frozenset(k for k in range(NSEG) if k % 6 == 3)  # segments on gpsimd

_CACHE = {}
LAST_EXEC_NS = None


def _apply_walrus_compat_patches():
    """This container's walrus rejects >1 sync-wait command per instruction;
    Tile's wait assignment can emit several. Split excess waits onto bare
    same-engine NoOps committed immediately before the instruction."""
    import concourse.tile as tile
    from concourse import mybir
    from concourse.vector_clock import ScopedClock

    if getattr(tile.TileContext, "_akima_patched", False):
        return
    MAX_WAITS = 1
    _orig_commit = tile.TileContext._commit_instruction

    def _split_waits(self, inst, lazy_reg_writes=True):
        si = inst.sync_info
        if si is not None and si.on_wait and len(si.on_wait) > MAX_WAITS:
            waits = list(si.on_wait)
            updates = list(si.on_update or [])
            inst.sync_info = mybir.SyncInfo(on_wait=waits[:MAX_WAITS], on_update=updates)
            for i in range(MAX_WAITS, len(waits), MAX_WAITS):
                nop = mybir.InstNoOp(name=f"I-{self.nc.next_id()}", engine=inst.engine)
                nop.sync_info = mybir.SyncInfo(on_wait=waits[i : i + MAX_WAITS], on_update=[])
                _orig_commit(self, nop, lazy_reg_writes)
        return _orig_commit(self, inst, lazy_reg_writes)

    def _drain_and_barrier(self, tick_clock, wait_clock):
        nc = self.nc
        collector = nc.sync.nop(nofuse=True).ins
        wait_clock.add_sem_waits(collector, ScopedClock({None: tick_clock.global_clock}))
        si = collector.sync_info
        waits = list(si.on_wait or []) if si is not None else []
        updates = list(si.on_update or []) if si is not None else []
        if len(waits) > MAX_WAITS:
            collector.sync_info = mybir.SyncInfo(on_wait=waits[:MAX_WAITS], on_update=updates)
            rest = waits[MAX_WAITS:]
            while rest:
                extra = nc.sync.nop(nofuse=True).ins
                extra.sync_info = mybir.SyncInfo(on_wait=rest[:MAX_WAITS], on_update=[])
                rest = rest[MAX_WAITS:]
        nc.sync.drain()
        nc.all_engine_barrier()
        assert self.sems is not None
        popped = nc._tile_sem_poison_stack.pop()
        assert popped is self._sem_poison
        nc.clear_and_free_semaphores(list(self.sems.allocated().values()))
        nc.all_engine_barrier()

    tile.TileContext._commit_instruction = _split_waits
    tile.TileContext._drain_and_barrier = _drain_and_barrier
    tile.TileContext._akima_patched = True


def _coefficients(value):
    """Per-segment cubic coefficients (A, B, C) and v0, in float64, following
    the reference Akima construction."""
    v = np.asarray(value, dtype=np.float64)
    n = v.shape[0]
    h = 1.0 / (n - 1)
    m = np.diff(v) / h
    m_ext = np.concatenate(
        [[3 * m[0] - 2 * m[1], 2 * m[0] - m[1]], m,
         [2 * m[-1] - m[-2], 3 * m[-1] - 2 * m[-2]]]
    )
    dm = np.abs(np.diff(m_ext))
    w1 = dm[2:]
    w2 = dm[:-2]
    den = w1 + w2
    safe = np.where(den > 0, den, 1.0)
    ml = m_ext[1 : n + 1]
    mr = m_ext[2 : n + 2]
    t = np.where(den > 0, (w1 * ml + w2 * mr) / safe, 0.5 * (ml + mr))
    A = (t[:-1] * h).astype(np.float32)
    Bc = ((3 * m - 2 * t[:-1] - t[1:]) * h).astype(np.float32)
    Cc = ((t[:-1] + t[1:] - 2 * m) * h).astype(np.float32)
    return A, Bc, Cc, np.float32(v[0])


def _build_bass(A, Bc, Cc, v0):
    import concourse.bass as bass
    import concourse.tile as tile
    from concourse import mybir

    AL = mybir.AluOpType
    F32 = mybir.dt.float32
    nc = bass.Bass()
    x = nc.declare_dram_parameter("x", [P, FTOT], F32, isOutput=False)
    y = nc.declare_dram_parameter("y", [P, FTOT], F32, isOutput=True)

    with tile.TileContext(nc) as tc:
        with (
            tc.tile_pool(name="xp", bufs=2) as xp,
            tc.tile_pool(name="Xp", bufs=2) as Xp,
            tc.tile_pool(name="up", bufs=3) as up,
            tc.tile_pool(name="pp", bufs=3) as pp,
            tc.tile_pool(name="gp", bufs=6) as gp,
            tc.tile_pool(name="ap", bufs=2) as ap,
        ):
            for it in range(NT):
                xt = xp.tile([P, TF], F32, tag="xt")
                nc.sync.dma_start(xt[:], x[:, bass.ts(it, TF)])
                Xt = Xp.tile([P, TF], F32, tag="Xt")
                nc.scalar.mul(Xt[:], xt[:], 63.0)
                acc = ap.tile([P, TF], F32, tag="acc")
                pend = []

                def emit_add(g):
                    pend.append(g)
                    if len(pend) == 2:
                        nc.gpsimd.tensor_tensor(out=acc[:], in0=pend[0][:], in1=pend[1][:], op=AL.add)
                        pend.clear()
                    elif acc_started[0]:
                        nc.gpsimd.tensor_tensor(out=acc[:], in0=acc[:], in1=pend[0][:], op=AL.add)
                        pend.clear()

                acc_started = [False]
                for k in range(NSEG):
                    a, b, c = float(A[k]), float(Bc[k]), float(Cc[k])
                    on_gp = k in GP_SEGS
                    eng = nc.gpsimd if on_gp else nc.vector
                    u2 = up.tile([P, TF], F32, tag="u2")
                    # u2 = min(X, k+1) - k ; then clamp low: d = max(u2, 0)
                    eng.tensor_scalar(out=u2[:], in0=Xt[:], scalar1=float(k + 1),
                                      scalar2=float(k), op0=AL.min, op1=AL.subtract)
                    eng.tensor_scalar(out=u2[:], in0=u2[:], scalar1=0.0, scalar2=None, op0=AL.max)
                    p = pp.tile([P, TF], F32, tag="p")
                    # p = d*C + B ; q = p*d ; g = (q + A)*d
                    eng.tensor_scalar(out=p[:], in0=u2[:], scalar1=c, scalar2=b, op0=AL.mult, op1=AL.add)
                    eng.tensor_tensor(out=p[:], in0=p[:], in1=u2[:], op=AL.mult)
                    g = gp.tile([P, TF], F32, tag="g")
                    if on_gp:
                        eng.tensor_scalar(out=p[:], in0=p[:], scalar1=a, scalar2=None, op0=AL.add)
                        eng.tensor_tensor(out=g[:], in0=p[:], in1=u2[:], op=AL.mult)
                    else:
                        eng.scalar_tensor_tensor(out=g[:], in0=p[:], scalar=a, in1=u2[:], op0=AL.add, op1=AL.mult)
                    emit_add(g)
                    if len(pend) == 0:
                        acc_started[0] = True

                yt = ap.tile([P, TF], F32, tag="yt")
                nc.vector.tensor_scalar(out=yt[:], in0=acc[:], scalar1=float(v0), scalar2=None, op0=AL.add)
                nc.sync.dma_start(y[:, bass.ts(it, TF)], yt[:])
    return nc


def kernel(input, value):
    global LAST_EXEC_NS
    import time

    _apply_walrus_compat_patches()
    from concourse.bass_utils import run_bass_kernel_spmd

    input = np.ascontiguousarray(np.asarray(input, dtype=np.float32))
    value = np.ascontiguousarray(np.asarray(value, dtype=np.float32))
    key = value.tobytes()
    nc = _CACHE.get(key)
    if nc is None:
        A, Bc, Cc, v0 = _coefficients(value)
        nc = _build_bass(A, Bc, Cc, v0)
        _CACHE.clear()
        _CACHE[key] = nc

    shards = input.reshape(N_CORES, P, FTOT)
    in_maps = [{"x": shards[c]} for c in range(N_CORES)]
    t0 = time.time()
    res = run_bass_kernel_spmd(nc, in_maps, core_ids=list(range(N_CORES)))
    LAST_EXEC_NS = (time.time() - t0) * 1e9
    out = np.stack([res.results[c]["y"] for c in range(N_CORES)], axis=0)
    return out.reshape(B, CH, H, W).astype(np.float32, copy=False)


# revision 3
# speedup vs baseline: 990.2317x; 990.2317x over previous
"""Akima spline evaluation (nn_Akima_66623532696299) on 8 Trainium2 cores.

Strategy: data-parallel over the batch axis (8 batches per core). Per element
the spline y(x) is evaluated without any gather via the telescoped identity

    y(x) = v0 + sum_k g_k(d_k),   d_k = clamp(63*x - k, 0, 1)
    g_k(d) = d*(A_k + d*(B_k + d*C_k))

where g_k is segment k's cubic expressed in the normalized local coordinate
(g_k(1) = v_{k+1} - v_k exactly, so partial sums stay O(|v|) and fp32
accumulation is stable). Segment coefficients are derived from `value` on the
host in float64 and baked into the instruction stream as immediates.

Engines: DVE (vector) computes most segments' cubics with fused
tensor_scalar / scalar_tensor_tensor ops; GPSIMD computes a subset of
segments and owns the serial accumulation chain; ACT handles the 63*x
prescale; HWDGE (sync) does the DMA. The work split is chosen so DVE and
GPSIMD finish together.
"""

import numpy as np

N_CORES = 8
P = 128
B, CH, H, W = 64, 3, 512, 512
PER_CORE = (B // N_CORES) * CH * H * W        # 6291456
FTOT = PER_CORE // P                          # 49152
TF = 2048                                     # tile free size
NT = FTOT // TF                               # 24 tiles
NSEG = 63
GP_SEGS = frozenset(k for k in range(NSEG) if k % 6 == 3)  # segments on gpsimd

_CACHE = {}
LAST_EXEC_NS = None


def _apply_walrus_compat_patches():
    """This container's walrus rejects >1 sync-wait command per instruction;
    Tile's wait assignment can emit several. Split excess waits onto bare
    same-engine NoOps committed immediately before the instruction."""
    import concourse.tile as tile
    from concourse import mybir
    from concourse.vector_clock import ScopedClock

    if getattr(tile.TileContext, "_akima_patched", False):
        return
    MAX_WAITS = 1
    _orig_commit = tile.TileContext._commit_instruction

    def _split_waits(self, inst, lazy_reg_writes=True):
        si = inst.sync_info
        if si is not None and si.on_wait and len(si.on_wait) > MAX_WAITS:
            waits = list(si.on_wait)
            updates = list(si.on_update or [])
            inst.sync_info = mybir.SyncInfo(on_wait=waits[:MAX_WAITS], on_update=updates)
            for i in range(MAX_WAITS, len(waits), MAX_WAITS):
                nop = mybir.InstNoOp(name=f"I-{self.nc.next_id()}", engine=inst.engine)
                nop.sync_info = mybir.SyncInfo(on_wait=waits[i : i + MAX_WAITS], on_update=[])
                _orig_commit(self, nop, lazy_reg_writes)
        return _orig_commit(self, inst, lazy_reg_writes)

    def _drain_and_barrier(self, tick_clock, wait_clock):
        nc = self.nc
        collector = nc.sync.nop(nofuse=True).ins
        wait_clock.add_sem_waits(collector, ScopedClock({None: tick_clock.global_clock}))
        si = collector.sync_info
        waits = list(si.on_wait or []) if si is not None else []
        updates = list(si.on_update or []) if si is not None else []
        if len(waits) > MAX_WAITS:
            collector.sync_info = mybir.SyncInfo(on_wait=waits[:MAX_WAITS], on_update=updates)
            rest = waits[MAX_WAITS:]
            while rest:
                extra = nc.sync.nop(nofuse=True).ins
                extra.sync_info = mybir.SyncInfo(on_wait=rest[:MAX_WAITS], on_update=[])
                rest = rest[MAX_WAITS:]
        nc.sync.drain()
        nc.all_engine_barrier()
        assert self.sems is not None
        popped = nc._tile_sem_poison_stack.pop()
        assert popped is self._sem_poison
        nc.clear_and_free_semaphores(list(self.sems.allocated().values()))
        nc.all_engine_barrier()

    tile.TileContext._commit_instruction = _split_waits
    tile.TileContext._drain_and_barrier = _drain_and_barrier
    tile.TileContext._akima_patched = True


def _coefficients(value):
    """Per-segment cubic coefficients (A, B, C) and v0, in float64, following
    the reference Akima construction."""
    v = np.asarray(value, dtype=np.float64)
    n = v.shape[0]
    h = 1.0 / (n - 1)
    m = np.diff(v) / h
    m_ext = np.concatenate(
        [[3 * m[0] - 2 * m[1], 2 * m[0] - m[1]], m,
         [2 * m[-1] - m[-2], 3 * m[-1] - 2 * m[-2]]]
    )
    dm = np.abs(np.diff(m_ext))
    w1 = dm[2:]
    w2 = dm[:-2]
    den = w1 + w2
    safe = np.where(den > 0, den, 1.0)
    ml = m_ext[1 : n + 1]
    mr = m_ext[2 : n + 2]
    t = np.where(den > 0, (w1 * ml + w2 * mr) / safe, 0.5 * (ml + mr))
    A = (t[:-1] * h).astype(np.float32)
    Bc = ((3 * m - 2 * t[:-1] - t[1:]) * h).astype(np.float32)
    Cc = ((t[:-1] + t[1:] - 2 * m) * h).astype(np.float32)
    return A, Bc, Cc, np.float32(v[0])


def _build_bass(A, Bc, Cc, v0, reps=1):
    import concourse.bass as bass
    import concourse.tile as tile
    from concourse import mybir

    AL = mybir.AluOpType
    F32 = mybir.dt.float32
    nc = bass.Bass()
    x = nc.declare_dram_parameter("x", [P, FTOT], F32, isOutput=False)
    y = nc.declare_dram_parameter("y", [P, FTOT], F32, isOutput=True)

    with tile.TileContext(nc) as tc:
        with (
            tc.tile_pool(name="xp", bufs=2) as xp,
            tc.tile_pool(name="Xp", bufs=2) as Xp,
            tc.tile_pool(name="up", bufs=3) as up,
            tc.tile_pool(name="pp", bufs=3) as pp,
            tc.tile_pool(name="gp", bufs=6) as gp,
            tc.tile_pool(name="ap", bufs=2) as ap,
        ):
            for it in [i % NT for i in range(NT * reps)]:
                xt = xp.tile([P, TF], F32, tag="xt")
                nc.sync.dma_start(xt[:], x[:, bass.ts(it, TF)])
                Xt = Xp.tile([P, TF], F32, tag="Xt")
                nc.scalar.mul(Xt[:], xt[:], 63.0)
                acc = ap.tile([P, TF], F32, tag="acc")
                first_g = [None]
                acc_started = [False]

                def emit_add(g):
                    if not acc_started[0]:
                        if first_g[0] is None:
                            first_g[0] = g
                        else:
                            nc.vector.tensor_tensor(out=acc[:], in0=first_g[0][:], in1=g[:], op=AL.add)
                            acc_started[0] = True
                    else:
                        nc.vector.tensor_tensor(out=acc[:], in0=acc[:], in1=g[:], op=AL.add)

                for k in range(NSEG):
                    a, b, c = float(A[k]), float(Bc[k]), float(Cc[k])
                    u2 = up.tile([P, TF], F32, tag="u2")
                    # u2 = min(X, k+1) - k ; then clamp low: d = max(u2, 0)
                    nc.vector.tensor_scalar(out=u2[:], in0=Xt[:], scalar1=float(k + 1),
                                            scalar2=float(k), op0=AL.min, op1=AL.subtract)
                    nc.vector.tensor_scalar(out=u2[:], in0=u2[:], scalar1=0.0, scalar2=None, op0=AL.max)
                    p = pp.tile([P, TF], F32, tag="p")
                    # p = d*C + B ; q = p*d ; g = (q + A)*d
                    nc.vector.tensor_scalar(out=p[:], in0=u2[:], scalar1=c, scalar2=b, op0=AL.mult, op1=AL.add)
                    nc.vector.tensor_tensor(out=p[:], in0=p[:], in1=u2[:], op=AL.mult)
                    g = gp.tile([P, TF], F32, tag="g")
                    nc.vector.scalar_tensor_tensor(out=g[:], in0=p[:], scalar=a, in1=u2[:], op0=AL.add, op1=AL.mult)
                    emit_add(g)

                yt = ap.tile([P, TF], F32, tag="yt")
                nc.vector.tensor_scalar(out=yt[:], in0=acc[:], scalar1=float(v0), scalar2=None, op0=AL.add)
                nc.sync.dma_start(y[:, bass.ts(it, TF)], yt[:])
    return nc


def kernel(input, value):
    global LAST_EXEC_NS
    import time

    _apply_walrus_compat_patches()
    from concourse.bass_utils import run_bass_kernel_spmd

    input = np.ascontiguousarray(np.asarray(input, dtype=np.float32))
    value = np.ascontiguousarray(np.asarray(value, dtype=np.float32))
    key = value.tobytes()
    nc = _CACHE.get(key)
    if nc is None:
        A, Bc, Cc, v0 = _coefficients(value)
        nc = _build_bass(A, Bc, Cc, v0)
        _CACHE.clear()
        _CACHE[key] = nc

    shards = input.reshape(N_CORES, P, FTOT)
    in_maps = [{"x": shards[c]} for c in range(N_CORES)]
    t0 = time.time()
    res = run_bass_kernel_spmd(nc, in_maps, core_ids=list(range(N_CORES)))
    LAST_EXEC_NS = (time.time() - t0) * 1e9
    out = np.stack([res.results[c]["y"] for c in range(N_CORES)], axis=0)
    return out.reshape(B, CH, H, W).astype(np.float32, copy=False)


# revision 4
# speedup vs baseline: 2216.2181x; 2.2381x over previous
"""Akima spline evaluation (nn_Akima_66623532696299) on 8 Trainium2 cores.

Strategy: data-parallel over the batch axis (8 batches per core). Per element
the spline y(x) is evaluated without any gather via the telescoped identity

    y(x) = v0 + sum_k g_k(d_k),   d_k = clamp(63*x - k, 0, 1)
    g_k(d) = d*(A_k + d*(B_k + d*C_k))

where g_k is segment k's cubic expressed in the normalized local coordinate
(g_k(1) = v_{k+1} - v_k exactly, so partial sums stay O(|v|) and fp32
accumulation is stable). Segment coefficients are derived from `value` on the
host in float64 and baked into the instruction stream as immediates.

Engines: DVE (vector) evaluates every segment with fused tensor_scalar /
scalar_tensor_tensor ops and owns the accumulation (measured fastest: the
dual-scalar tensor_scalar ops run in the DVE 2x perf mode and cross-engine
handoffs cost more than they save); ACT handles the 63*x prescale; HWDGE
(sync) does the DMA, overlapped via double-buffered tile pools.
"""

import numpy as np

N_CORES = 8
P = 128
B, CH, H, W = 64, 3, 512, 512
PER_CORE = (B // N_CORES) * CH * H * W        # 6291456
FTOT = PER_CORE // P                          # 49152
TF = 2048                                     # tile free size
NT = FTOT // TF                               # 24 tiles
NSEG = 63

_CACHE = {}
LAST_EXEC_NS = None


def _apply_walrus_compat_patches():
    """This container's walrus rejects >1 sync-wait command per instruction;
    Tile's wait assignment can emit several. Split excess waits onto bare
    same-engine NoOps committed immediately before the instruction."""
    import concourse.tile as tile
    from concourse import mybir
    from concourse.vector_clock import ScopedClock

    if getattr(tile.TileContext, "_akima_patched", False):
        return
    MAX_WAITS = 1
    _orig_commit = tile.TileContext._commit_instruction

    def _split_waits(self, inst, lazy_reg_writes=True):
        si = inst.sync_info
        if si is not None and si.on_wait and len(si.on_wait) > MAX_WAITS:
            waits = list(si.on_wait)
            updates = list(si.on_update or [])
            inst.sync_info = mybir.SyncInfo(on_wait=waits[:MAX_WAITS], on_update=updates)
            for i in range(MAX_WAITS, len(waits), MAX_WAITS):
                nop = mybir.InstNoOp(name=f"I-{self.nc.next_id()}", engine=inst.engine)
                nop.sync_info = mybir.SyncInfo(on_wait=waits[i : i + MAX_WAITS], on_update=[])
                _orig_commit(self, nop, lazy_reg_writes)
        return _orig_commit(self, inst, lazy_reg_writes)

    def _drain_and_barrier(self, tick_clock, wait_clock):
        nc = self.nc
        collector = nc.sync.nop(nofuse=True).ins
        wait_clock.add_sem_waits(collector, ScopedClock({None: tick_clock.global_clock}))
        si = collector.sync_info
        waits = list(si.on_wait or []) if si is not None else []
        updates = list(si.on_update or []) if si is not None else []
        if len(waits) > MAX_WAITS:
            collector.sync_info = mybir.SyncInfo(on_wait=waits[:MAX_WAITS], on_update=updates)
            rest = waits[MAX_WAITS:]
            while rest:
                extra = nc.sync.nop(nofuse=True).ins
                extra.sync_info = mybir.SyncInfo(on_wait=rest[:MAX_WAITS], on_update=[])
                rest = rest[MAX_WAITS:]
        nc.sync.drain()
        nc.all_engine_barrier()
        assert self.sems is not None
        popped = nc._tile_sem_poison_stack.pop()
        assert popped is self._sem_poison
        nc.clear_and_free_semaphores(list(self.sems.allocated().values()))
        nc.all_engine_barrier()

    tile.TileContext._commit_instruction = _split_waits
    tile.TileContext._drain_and_barrier = _drain_and_barrier
    tile.TileContext._akima_patched = True


def _coefficients(value):
    """Per-segment cubic coefficients (A, B, C) and v0, in float64, following
    the reference Akima construction."""
    v = np.asarray(value, dtype=np.float64)
    n = v.shape[0]
    h = 1.0 / (n - 1)
    m = np.diff(v) / h
    m_ext = np.concatenate(
        [[3 * m[0] - 2 * m[1], 2 * m[0] - m[1]], m,
         [2 * m[-1] - m[-2], 3 * m[-1] - 2 * m[-2]]]
    )
    dm = np.abs(np.diff(m_ext))
    w1 = dm[2:]
    w2 = dm[:-2]
    den = w1 + w2
    safe = np.where(den > 0, den, 1.0)
    ml = m_ext[1 : n + 1]
    mr = m_ext[2 : n + 2]
    t = np.where(den > 0, (w1 * ml + w2 * mr) / safe, 0.5 * (ml + mr))
    A = (t[:-1] * h).astype(np.float32)
    Bc = ((3 * m - 2 * t[:-1] - t[1:]) * h).astype(np.float32)
    Cc = ((t[:-1] + t[1:] - 2 * m) * h).astype(np.float32)
    return A, Bc, Cc, np.float32(v[0])


def _build_bass(A, Bc, Cc, v0, reps=1):
    import concourse.bass as bass
    import concourse.tile as tile
    from concourse import mybir

    AL = mybir.AluOpType
    F32 = mybir.dt.float32
    nc = bass.Bass()
    x = nc.declare_dram_parameter("x", [P, FTOT], F32, isOutput=False)
    y = nc.declare_dram_parameter("y", [P, FTOT], F32, isOutput=True)

    with tile.TileContext(nc) as tc:
        with (
            tc.tile_pool(name="xp", bufs=2) as xp,
            tc.tile_pool(name="Xp", bufs=2) as Xp,
            tc.tile_pool(name="up", bufs=3) as up,
            tc.tile_pool(name="pp", bufs=3) as pp,
            tc.tile_pool(name="gp", bufs=6) as gp,
            tc.tile_pool(name="ap", bufs=2) as ap,
        ):
            for it in [i % NT for i in range(NT * reps)]:
                xt = xp.tile([P, TF], F32, tag="xt")
                nc.sync.dma_start(xt[:], x[:, bass.ts(it, TF)])
                Xt = Xp.tile([P, TF], F32, tag="Xt")
                nc.scalar.mul(Xt[:], xt[:], 63.0)
                acc = ap.tile([P, TF], F32, tag="acc")
                first_g = [None]
                acc_started = [False]

                def emit_add(g):
                    if not acc_started[0]:
                        if first_g[0] is None:
                            first_g[0] = g
                        else:
                            nc.vector.tensor_tensor(out=acc[:], in0=first_g[0][:], in1=g[:], op=AL.add)
                            acc_started[0] = True
                    else:
                        nc.vector.tensor_tensor(out=acc[:], in0=acc[:], in1=g[:], op=AL.add)

                for k in range(NSEG):
                    a, b, c = float(A[k]), float(Bc[k]), float(Cc[k])
                    u2 = up.tile([P, TF], F32, tag="u2")
                    # u2 = min(X, k+1) - k ; then clamp low: d = max(u2, 0)
                    nc.vector.tensor_scalar(out=u2[:], in0=Xt[:], scalar1=float(k + 1),
                                            scalar2=float(k), op0=AL.min, op1=AL.subtract)
                    nc.vector.tensor_scalar(out=u2[:], in0=u2[:], scalar1=0.0, scalar2=None, op0=AL.max)
                    p = pp.tile([P, TF], F32, tag="p")
                    # p = d*C + B ; q = p*d ; g = (q + A)*d
                    nc.vector.tensor_scalar(out=p[:], in0=u2[:], scalar1=c, scalar2=b, op0=AL.mult, op1=AL.add)
                    nc.vector.tensor_tensor(out=p[:], in0=p[:], in1=u2[:], op=AL.mult)
                    g = gp.tile([P, TF], F32, tag="g")
                    nc.vector.scalar_tensor_tensor(out=g[:], in0=p[:], scalar=a, in1=u2[:], op0=AL.add, op1=AL.mult)
                    emit_add(g)

                yt = ap.tile([P, TF], F32, tag="yt")
                nc.vector.tensor_scalar(out=yt[:], in0=acc[:], scalar1=float(v0), scalar2=None, op0=AL.add)
                nc.sync.dma_start(y[:, bass.ts(it, TF)], yt[:])
    return nc


def kernel(input, value):
    global LAST_EXEC_NS
    import time

    _apply_walrus_compat_patches()
    from concourse.bass_utils import run_bass_kernel_spmd

    input = np.ascontiguousarray(np.asarray(input, dtype=np.float32))
    value = np.ascontiguousarray(np.asarray(value, dtype=np.float32))
    key = value.tobytes()
    nc = _CACHE.get(key)
    if nc is None:
        A, Bc, Cc, v0 = _coefficients(value)
        nc = _build_bass(A, Bc, Cc, v0)
        _CACHE.clear()
        _CACHE[key] = nc

    shards = input.reshape(N_CORES, P, FTOT)
    in_maps = [{"x": shards[c]} for c in range(N_CORES)]
    t0 = time.time()
    res = run_bass_kernel_spmd(nc, in_maps, core_ids=list(range(N_CORES)))
    LAST_EXEC_NS = (time.time() - t0) * 1e9
    out = np.stack([res.results[c]["y"] for c in range(N_CORES)], axis=0)
    return out.reshape(B, CH, H, W).astype(np.float32, copy=False)
